# revision 6
# baseline (speedup 1.0000x reference)
"""APPNP (MLP encoder + K-step propagation + log_softmax) on 8 TRN2 cores.

Self-contained; hardcodes shapes:
  x [100000,512] f32, edge_index [2,3200000] int, W1 [512,64], b1 [64],
  W2 [64,40], b2 [40]  ->  log_softmax(z_K) [100000,40] f32.

Math (y-space -- no per-edge weights needed):
  z_{t+1} = (1-a) D^-1/2 (A+I) D^-1/2 z_t + a h       (A_hat, self-loops)
  with y = D^-1/2 z:   y_{t+1} = (1-a) D^-1 (y + A^T-sum) + a*hs
  where agg[v] = sum_{u->v} y[u], hs = D^-1/2 h, z_K = D^1/2 y_K.

Mapping: nodes sharded 12500/core (padded 12544 = 128*98). Node local id i
sits at SBUF (partition i%128, col i//128); flat DRAM row perm(i) =
(i%128)*98 + i//128. Per step each core:
  - dma_gather y[src] rows (256B) from the replicated y table, 1024
    idx/call (hardware faults above ~1024), int16 idx -> 4 chunks of
    25088 rows,
  - dma_scatter_add into its agg rows; every call touches each agg row at
    most once (host ranks edges per (chunk,dst) -- the SDMA CCE add loses
    colliding updates within a call),
  - y_new = (1-a)*dinv2*(y+agg) + a*hs on DVE, AllGather shards -> table.
"""
import os
import sys
import numpy as np

for _p in ('/opt/trn_rl_repo', '/root/.axon_site/_ro/trn_rl_repo'):
    if _p not in sys.path and os.path.isdir(_p):
        sys.path.insert(0, _p)

N = 100000
F = 512
HID = 64
C = 40
K = 10
ALPHA = 0.1
NCORES = 8
SH = 12500
SHP = 12544          # 128*98
NPC = 98
NT = SHP * NCORES    # 100352
CHUNK = NT // 4      # 25088 < 32768
ROW = 64             # f32 row -> 256B
CALL = 1024
AGG_ROWS = 12800     # 12544 real + trash
TRASH = 12799
P = 128
FREE = NPC * ROW     # 6272

_CACHE = {}


def _wrap_rep(idx1024):
    w = idx1024.reshape(64, 16).T.astype(np.int16)
    return np.tile(w, (8, 1))          # [128, 64]


def _perm(i):
    return (i % P) * NPC + i // P


def _preprocess(edge_index):
    src = np.asarray(edge_index[0], dtype=np.int64)
    dst = np.asarray(edge_index[1], dtype=np.int64)
    deg = np.bincount(dst, minlength=N).astype(np.float64) + 1.0

    core_of = dst // SH
    tab_row = (src // SH) * SHP + _perm(src % SH)
    chunk = tab_row // CHUNK
    lidx = tab_row % CHUNK

    percore = []
    for c in range(NCORES):
        m = core_of == c
        q, li = chunk[m], lidx[m]
        dl = _perm(dst[m] % SH)
        o1 = np.lexsort((dl, q))
        q, dl, li = q[o1], dl[o1], li[o1]
        n = len(q)
        if n:
            new = np.r_[True, (q[1:] != q[:-1]) | (dl[1:] != dl[:-1])]
            gstart = np.maximum.accumulate(np.where(new, np.arange(n), 0))
            rank = np.arange(n) - gstart
        else:
            rank = np.zeros(0, np.int64)
        o2 = np.lexsort((dl, rank, q))
        q, rank, dl, li = q[o2], rank[o2], dl[o2], li[o2]
        calls_q = [[] for _ in range(4)]
        for qq in range(4):
            mq = q == qq
            rq, dq, lq = rank[mq], dl[mq], li[mq]
            if len(rq):
                bnd = np.r_[0, np.where(rq[1:] != rq[:-1])[0] + 1, len(rq)]
            else:
                bnd = np.array([0, 0])
            for b in range(len(bnd) - 1):
                s, e = int(bnd[b]), int(bnd[b + 1])
                for s2 in range(s, e, CALL):
                    e2 = min(s2 + CALL, e)
                    g = np.zeros(CALL, np.int64)
                    sc = np.full(CALL, TRASH, np.int64)
                    g[: e2 - s2] = lq[s2:e2]
                    sc[: e2 - s2] = dq[s2:e2]
                    calls_q[qq].append((g, sc))
        percore.append(calls_q)

    ncalls_q = [max(len(percore[c][qq]) for c in range(NCORES)) for qq in range(4)]
    total = sum(ncalls_q)
    dummy_g = np.zeros(CALL, np.int64)
    dummy_s = np.full(CALL, TRASH, np.int64)
    gidx_all, sidx_all = [], []
    for c in range(NCORES):
        gs, ss = [], []
        for qq in range(4):
            lst = percore[c][qq]
            for j in range(ncalls_q[qq]):
                g, s = lst[j] if j < len(lst) else (dummy_g, dummy_s)
                gs.append(_wrap_rep(g))
                ss.append(_wrap_rep(s))
        gidx_all.append(np.concatenate(gs, axis=1))
        sidx_all.append(np.concatenate(ss, axis=1))
    return gidx_all, sidx_all, ncalls_q, total, deg


def _build(ncalls_q, total):
    from concourse import bacc, mybir
    import concourse.tile as tile

    dt = mybir.dt
    AF = mybir.ActivationFunctionType
    nc = bacc.Bacc("TRN2", target_bir_lowering=False, debug=False,
                   num_devices=NCORES)

    xT_ext = nc.dram_tensor("xT", [F, SHP], dt.float32, kind="ExternalInput")
    w1_ext = nc.dram_tensor("w1", [P, 4 * HID], dt.float32, kind="ExternalInput")
    b1_ext = nc.dram_tensor("b1", [HID, 1], dt.float32, kind="ExternalInput")
    w2_ext = nc.dram_tensor("w2", [HID, C], dt.float32, kind="ExternalInput")
    b2_ext = nc.dram_tensor("b2", [C, 1], dt.float32, kind="ExternalInput")
    gidx_ext = nc.dram_tensor("gidx", [P, total * 64], dt.int16, kind="ExternalInput")
    sidx_ext = nc.dram_tensor("sidx", [P, total * 64], dt.int16, kind="ExternalInput")
    dinv_ext = nc.dram_tensor("dinvr", [P, FREE], dt.float32, kind="ExternalInput")
    s1_ext = nc.dram_tensor("s1r", [P, FREE], dt.float32, kind="ExternalInput")
    dsq_ext = nc.dram_tensor("dsqr", [P, FREE], dt.float32, kind="ExternalInput")
    zero_ext = nc.dram_tensor("zero", [AGG_ROWS, ROW], dt.float32, kind="ExternalInput")
    out_ext = nc.dram_tensor("out", [C, SHP], dt.float32, kind="ExternalOutput")

    NTILE = 512
    tiles = [(i * NTILE, min(NTILE, SHP - i * NTILE))
             for i in range((SHP + NTILE - 1) // NTILE)]
    GRP = 8

    from concourse.masks import make_identity

    with tile.TileContext(nc) as tc:
        with tc.tile_pool(name="res", bufs=1) as resp, \
             tc.tile_pool(name="mlp", bufs=3) as mlpp, \
             tc.tile_pool(name="ps", bufs=1, space="PSUM") as psp, \
             tc.tile_pool(name="msg", bufs=6) as msgp, \
             tc.tile_pool(name="idx", bufs=3) as idxp, \
             tc.tile_pool(name="dram", bufs=1, space="DRAM") as dramp:

            y_tab = dramp.tile([NT, ROW], dt.float32)
            y_own = dramp.tile([SHP, ROW], dt.float32)
            agg = dramp.tile([AGG_ROWS, ROW], dt.float32)

            w1_sb = resp.tile([P, 4 * HID], dt.float32)
            b1_sb = resp.tile([HID, 1], dt.float32)
            w2_sb = resp.tile([HID, C], dt.float32)
            b2_sb = resp.tile([C, 1], dt.float32)
            dinv_sb = resp.tile([P, FREE], dt.float32)
            s1_sb = resp.tile([P, FREE], dt.float32)
            hs_sb = resp.tile([P, FREE], dt.float32)
            y_sb = resp.tile([P, FREE], dt.float32)
            ident = resp.tile([P, P], dt.float32)
            ident40 = resp.tile([C, C], dt.float32)
            ones_sb = resp.tile([C, C], dt.float32)

            nc.sync.dma_start(out=w1_sb[:], in_=w1_ext.ap()[:])
            nc.sync.dma_start(out=b1_sb[:], in_=b1_ext.ap()[:])
            nc.sync.dma_start(out=w2_sb[:], in_=w2_ext.ap()[:])
            nc.sync.dma_start(out=b2_sb[:], in_=b2_ext.ap()[:])
            nc.sync.dma_start(out=dinv_sb[:], in_=dinv_ext.ap()[:])
            nc.sync.dma_start(out=s1_sb[:], in_=s1_ext.ap()[:])
            make_identity(nc, ident[:])
            make_identity(nc, ident40[:])
            nc.vector.memset(hs_sb[:], 0.0)
            nc.vector.memset(ones_sb[:], 1.0)

            # ---- MLP (transposed): zt_sb = hT = W2^T relu(W1^T x + b1) + b2
            for (t0, tn) in tiles:
                xt = mlpp.tile([P, 4 * NTILE], dt.float32, name="xt")
                for kk in range(4):
                    nc.sync.dma_start(
                        out=xt[:, kk * NTILE:kk * NTILE + tn],
                        in_=xT_ext.ap()[kk * P:(kk + 1) * P, t0:t0 + tn])
                ps1 = psp.tile([HID, NTILE], dt.float32, name="ps1", bufs=2)
                for kk in range(4):
                    nc.tensor.matmul(ps1[:, :tn],
                                     w1_sb[:, kk * HID:(kk + 1) * HID],
                                     xt[:, kk * NTILE:kk * NTILE + tn],
                                     start=(kk == 0), stop=(kk == 3))
                ht = mlpp.tile([HID, NTILE], dt.float32, name="ht")
                nc.scalar.activation(ht[:, :tn], ps1[:, :tn], AF.Relu,
                                     bias=b1_sb[:])
                ps2 = psp.tile([C, NTILE], dt.float32, name="ps2", bufs=2)
                nc.tensor.matmul(ps2[:, :tn], w2_sb[:], ht[:, :tn],
                                 start=True, stop=True)
                ht2 = mlpp.tile([C, NTILE], dt.float32, name="ht2")
                nc.scalar.activation(ht2[:, :tn], ps2[:, :tn],
                                     AF.Identity, bias=b2_sb[:])
                # transpose hT -> row layout: node j*128+p -> (p, col j)
                for u in range(tn // P):
                    j = t0 // P + u
                    pst = psp.tile([P, C], dt.float32, name="pst", bufs=2)
                    nc.tensor.transpose(out=pst[:],
                                        in_=ht2[:, u * P:(u + 1) * P],
                                        identity=ident40[:])
                    nc.vector.tensor_copy(out=hs_sb[:, j * ROW:j * ROW + C],
                                          in_=pst[:])

            # y0 = dinv*h ; hs = alpha*y0
            nc.vector.tensor_mul(out=y_sb[:], in0=hs_sb[:], in1=dinv_sb[:])
            nc.vector.tensor_scalar_mul(out=hs_sb[:], in0=y_sb[:], scalar1=ALPHA)

            yown_r = y_own[:, :].rearrange("(p j) d -> p (j d)", p=P)
            agg_r = agg[0:SHP, :].rearrange("(p j) d -> p (j d)", p=P)

            def allgather():
                nc.sync.dma_start(out=yown_r, in_=y_sb[:])
                nc.gpsimd.collective_compute(
                    "AllGather", mybir.AluOpType.bypass,
                    replica_groups=[list(range(NCORES))],
                    ins=[y_own[:, :].opt()],
                    outs=[y_tab[:, :].opt()],
                )

            allgather()

            for step in range(K):
                nc.sync.dma_start(out=agg[:, :], in_=zero_ext.ap()[:])
                ci = 0
                gi = si = None
                for qq in range(4):
                    base = qq * CHUNK
                    for j in range(ncalls_q[qq]):
                        if ci % GRP == 0:
                            ng = min(GRP, total - ci)
                            gi = idxp.tile([P, GRP * 64], dt.int16, name="gi")
                            si = idxp.tile([P, GRP * 64], dt.int16, name="si")
                            nc.sync.dma_start(
                                out=gi[:, :ng * 64],
                                in_=gidx_ext.ap()[:, ci * 64:(ci + ng) * 64])
                            nc.sync.dma_start(
                                out=si[:, :ng * 64],
                                in_=sidx_ext.ap()[:, ci * 64:(ci + ng) * 64])
                        o = ci % GRP
                        msg = msgp.tile([P, CALL // P, ROW], dt.float32,
                                        name="msg")
                        nc.gpsimd.dma_gather(
                            msg[:, :, :], y_tab[base:base + CHUNK, :],
                            gi[:, o * 64:(o + 1) * 64],
                            num_idxs=CALL, num_idxs_reg=CALL, elem_size=ROW)
                        nc.gpsimd.dma_scatter_add(
                            agg[:, :], msg[:, :, :],
                            si[:, o * 64:(o + 1) * 64],
                            num_idxs=CALL, num_idxs_reg=CALL, elem_size=ROW)
                        ci += 1
                # y = (1-a)*dinv2*(y + agg) + alpha*hs
                ag_sb = msgp.tile([P, FREE], dt.float32, name="ag_sb", bufs=1)
                nc.sync.dma_start(out=ag_sb[:], in_=agg_r)
                nc.vector.tensor_add(out=ag_sb[:], in0=ag_sb[:], in1=y_sb[:])
                nc.vector.tensor_mul(out=ag_sb[:], in0=ag_sb[:], in1=s1_sb[:])
                nc.vector.tensor_add(out=y_sb[:], in0=ag_sb[:], in1=hs_sb[:])
                if step < K - 1:
                    allgather()

            # ---- z = dsqrt*y ; log_softmax along classes (transposed layout)
            nc.sync.dma_start(out=hs_sb[:], in_=dsq_ext.ap()[:])
            nc.vector.tensor_mul(out=y_sb[:], in0=y_sb[:], in1=hs_sb[:])
            for (t0, tn) in tiles:
                zt = mlpp.tile([C, NTILE], dt.float32, name="zt")
                for u in range(tn // P):
                    j = t0 // P + u
                    psz = psp.tile([C, P], dt.float32, name="psz")
                    nc.tensor.transpose(out=psz[:],
                                        in_=y_sb[:, j * ROW:j * ROW + C],
                                        identity=ident[:])
                    nc.vector.tensor_copy(out=zt[:, u * P:(u + 1) * P],
                                          in_=psz[:])
                et = mlpp.tile([C, NTILE], dt.float32, name="et")
                nc.scalar.activation(et[:, :tn], zt[:, :tn], AF.Exp)
                pss = psp.tile([C, NTILE], dt.float32, name="pss")
                nc.tensor.matmul(pss[:, :tn], ones_sb[:], et[:, :tn],
                                 start=True, stop=True)
                lse = mlpp.tile([C, NTILE], dt.float32, name="lse")
                nc.scalar.activation(lse[:, :tn], pss[:, :tn], AF.Ln)
                ot = mlpp.tile([C, NTILE], dt.float32, name="ot")
                nc.vector.tensor_sub(out=ot[:, :tn],
                                     in0=zt[:, :tn],
                                     in1=lse[:, :tn])
                nc.sync.dma_start(out=out_ext.ap()[:, t0:t0 + tn],
                                  in_=ot[:, :tn])

    nc.compile()
    return nc


def kernel(x, edge_index, W1, b1, W2, b2):
    x = np.asarray(x, np.float32)
    W1 = np.asarray(W1, np.float32)
    b1 = np.asarray(b1, np.float32)
    W2 = np.asarray(W2, np.float32)
    b2 = np.asarray(b2, np.float32)

    gidx_all, sidx_all, ncalls_q, total, deg = _preprocess(edge_index)

    key = tuple(ncalls_q)
    if key not in _CACHE:
        _CACHE[key] = _build(ncalls_q, total)
    nc = _CACHE[key]

    dinv = (1.0 / np.sqrt(deg)).astype(np.float32)
    dinv2 = (1.0 / deg).astype(np.float32)
    dsq = np.sqrt(deg).astype(np.float32)

    def rep(vec_core):
        # node i=j*128+p -> [p, j*ROW:(j+1)*ROW] replicated across ROW
        v = np.zeros(SHP, np.float32)
        v[:SH] = vec_core
        r = v.reshape(NPC, P).T            # [P, NPC]
        return np.repeat(r[:, :, None], ROW, axis=2).reshape(P, FREE)

    w1_arr = W1.reshape(4, P, HID).transpose(1, 0, 2).reshape(P, 4 * HID).copy()
    zeros_agg = np.zeros((AGG_ROWS, ROW), np.float32)

    in_maps = []
    for c in range(NCORES):
        sl = slice(c * SH, (c + 1) * SH)
        xT = np.zeros((F, SHP), np.float32)
        xT[:, :SH] = x[sl].T
        in_maps.append({
            "xT": xT,
            "w1": w1_arr,
            "b1": b1.reshape(HID, 1),
            "w2": W2,
            "b2": b2.reshape(C, 1),
            "gidx": gidx_all[c],
            "sidx": sidx_all[c],
            "dinvr": rep(dinv[sl]),
            "s1r": rep((1.0 - ALPHA) * dinv2[sl]),
            "dsqr": rep(dsq[sl]),
            "zero": zeros_agg,
        })

    from concourse.bass_utils import run_bass_kernel_spmd
    trace = bool(os.environ.get("KERNEL_TRACE"))
    if trace:
        try:
            import bench_common
            bench_common.install_profile_hook()
        except Exception:
            trace = False
    res = run_bass_kernel_spmd(nc, in_maps, core_ids=list(range(NCORES)),
                               trace=trace)
    if trace and res.exec_time_ns is not None:
        print(f"HW exec time: {res.exec_time_ns} ns")

    out = np.empty((N, C), np.float32)
    for c in range(NCORES):
        out[c * SH:(c + 1) * SH] = res.results[c]["out"][:, :SH].T
    return out
